# revision 31
# baseline (speedup 1.0000x reference)
"""Locally-connected layer (no weight sharing) on 8 Trainium2 NeuronCores.

Problem: x (32,32,64,64) f32, weights (64,32,62,62,3,3) f32, biases (64,62,62).
out[b,o,i,j] = sum_{c,u,v} x[b,c,i+u,j+v] * w[o,c,i,j,u,v] + bias[o,i,j]

Strategy:
- Shard output rows i (OH=62 padded to 64) across 8 cores: core c computes
  rows [8c, 8c+8). Padded rows use zero weights and are dropped on host.
- Weights are the DMA hog (283 MB fp32). Ship them as fp8 e3m4 (1 byte):
  measured end-to-end rel_inf error ~1.4% < 2e-2 gate with x kept fp16.
- Matmul orientation: stationary lhsT = W[(u,c)=96, o=64] (fp8e3), moving
  rhs = X[(u,c)=96, b=32] (fp16), PSUM out [64 o, 32 b], v accumulated in
  PSUM. Streaming B=32 instead of O=64 halves PE time; LD_WEIGHTS is free.
- j-pair packing: even j at PE columns 0-63, odd j at 64-127
  (tile_position), so a PSUM bank [128, 512] holds 16 j-pairs x 32 b.
- x-row sharing: contraction partitions are u-major (p = u*32 + c), so an
  SBUF tile [128=(4 x-rows, c), W*B] serves output rows 2t (partition
  offset 0) and 2t+1 (offset 32). Tile t's bottom half is copied on-chip
  from tile t-1's top half; only 10 distinct x rows are DMA'd per core.
- Queue/tail engineering: HWDGE generation is a serial ~625ns/DMA shared
  resource -> weights ride SP one chunk per mid row; outputs ride SWDGE
  (Pool). PSUM->SBUF copies run on the otherwise idle Activation engine.
  Row 7 is reordered (bank 1 first) with descending weight chunks and
  split into five pipelined j-groups, each with its own PSUM tile
  (dependency tracking is tile-granular; a shared tile would serialize
  matmuls behind the previous group's copy), with copies alternating
  ACT/DVE and out DMAs spread over ACT/Pool/SP so the final dependency
  chain (last weight chunk -> 4 matmuls -> tiny copy -> tiny out DMA on
  SP) is as short as possible. wv_bufs=5 (swept: 4 and 6+ are slower) and
  fine row-0 weight chunks matter: the PE trails the DMA stream closely,
  so a late PE start or a buffer-recycle stall propagates to the tail.

  Measured on device: 38328 ns (TimelineSim), rel_inf err 1.39e-2
  (baseline fp16 kernel: 71797 ns). DMA-bound: 35.0us transfer floor +
  1.97us fixed head + ~1.8us tail chain/drains.
"""

import numpy as np

B, C, O = 32, 32, 64
H = W = 64
KK = 3
OH = OW = 62
NCORES = 8
RPC = 8  # output rows per core
PADH = NCORES * RPC  # 64
KP = 96  # contraction per matmul: (u, c)
XF = W * B  # x free size per row: s*32 + b, s in [0, 64)
WF = OW * 3 * O  # weight free size: j * 192 + v * 64 + o
NXROW = RPC + 2  # distinct x rows per core
OUTF = (OW // 2) * B  # 31 pairs * 32 b = 992

TRACE = False
LAST_RESULT = {}

CFG = {
    "wv_bufs": 5,
    "xpool_bufs": 1,
    "ps_bufs": 7,
    "xa1_s": 34,  # s-columns in the first x piece (covers all of bank 0)
    "xa2_s": 34,  # == xa1_s: second head piece elided (swept optimum)
    "w0_jc": 8,  # row-0 weight chunking
    "midchunks": 1,  # weight DMAs per mid row
    "l_chunks": [(32, 48), (48, 62), (0, 16), (16, 28), (28, 32)],
    "l_pieces": [  # (j0, j1, copy engine, out-dma (f0, f1, queue) or None)
        (32, 48, "act", None),
        (48, 62, "dve", (512, 992, "pool")),
        (0, 16, "act", (0, 256, "act")),
        (16, 28, "dve", None),
        (28, 32, "dve", (256, 512, "sp")),
    ],
}

_NC_CACHE = {}


def _build_nc():
    import concourse.bacc as bacc
    import concourse.mybir as mybir
    import concourse.tile as tile

    f16 = mybir.dt.float16
    f32 = mybir.dt.float32
    f8 = mybir.dt.float8e3

    nc = bacc.Bacc("TRN2", target_bir_lowering=False, debug=False)

    # x rows r=0..9 (global c0*8+r), layout [r][(c), s*32+b] on 32 partitions
    xv = nc.dram_tensor("xv", (NXROW, C, XF), f16, kind="ExternalInput")
    wv = nc.dram_tensor("wv", (RPC, KP, WF), f8, kind="ExternalInput")
    out_d = nc.dram_tensor("out", (RPC, 128, OUTF), f16, kind="ExternalOutput")

    with tile.TileContext(nc) as tc:
        with (
            tc.tile_pool(name="wpool", bufs=CFG["wv_bufs"]) as wpool,
            tc.tile_pool(name="xpool", bufs=CFG.get("xpool_bufs", 4)) as xpool,
            tc.tile_pool(name="opool", bufs=RPC) as opool,
            tc.tile_pool(name="pspool", bufs=CFG["ps_bufs"], space="PSUM") as pspool,
        ):
            # --- x tiles: allocate all 4 up front, prefetch DMAs early ---
            # Shared tile t holds x rows 2t..2t+3 (u-major blocks); even row
            # 2t reads partitions [0:96] directly. Odd rows cannot read
            # [32:128] (BIR: >32-partition spans must start at 0 or 64), so
            # each gets a dedicated [96, XF] tile filled by three
            # 32-partition DVE copies from the shared tile.
            x4 = []
            xo = []
            for t in range(4):
                x4_t = xpool.tile([128, XF], f16, tag=f"x4_{t}", name=f"x4_{t}")
                x4.append(x4_t)
                xo_t = xpool.tile([96, XF], f16, tag=f"xo_{t}", name=f"xo_{t}")
                xo.append(xo_t)
            xflat = xv[0:4].rearrange("r c f -> (r c) f")
            s1, s2 = CFG["xa1_s"] * 32, CFG["xa2_s"] * 32
            # head pieces on SP (fast HWDGE path) so the PE can start early
            nc.sync.dma_start(x4[0][:, :s1], xflat[:, :s1])
            if s2 > s1:
                nc.sync.dma_start(x4[0][:, s1:s2], xflat[:, s1:s2])
            # rest of tile 0 + top halves of tiles 1-3 on SWDGE
            nc.gpsimd.dma_start(x4[0][:, s2:], xflat[:, s2:])
            for t in range(1, 4):
                nc.gpsimd.dma_start(
                    x4[t][64:128, :],
                    xv[2 * t + 2 : 2 * t + 4].rearrange("r c f -> (r c) f"),
                )

            wv_t = None

            def do_bank(ps, js, xt, fbase):
                """Matmuls for j in js accumulating into psum tile ps."""
                for j in js:
                    dd = j & 1
                    fc = (j >> 1) * 32 - fbase
                    for v in range(3):
                        nc.tensor.matmul(
                            ps[64 * dd : 64 * dd + 64, fc : fc + 32],
                            wv_t[:, j * 192 + v * 64 : j * 192 + v * 64 + 64],
                            xt[0:96, (j + v) * 32 : (j + v) * 32 + 32],
                            start=(v == 0),
                            stop=(v == 2),
                            tile_position=(0, 64 * dd),
                        )

            for t in range(4):
                if t > 0:
                    # bottom half = prev tile's top half (on-chip)
                    nc.vector.tensor_copy(x4[t][0:64, :], x4[t - 1][64:128, :])
                for u in range(3):
                    nc.vector.tensor_copy(
                        xo[t][32 * u : 32 * u + 32, :],
                        x4[t][32 * u + 32 : 32 * u + 64, :],
                    )

                for d in range(2):
                    i = 2 * t + d
                    xsrc = x4[t] if d == 0 else xo[t]
                    last = i == RPC - 1
                    wv_t = wpool.tile([KP, WF], f8, tag="wv")
                    if i == 0:
                        jc = CFG["w0_jc"]
                        chunks = [(j0, min(j0 + jc, OW)) for j0 in range(0, OW, jc)]
                    elif last:
                        # bank 1 first, then bank 0 with a tiny final chunk
                        chunks = CFG["l_chunks"]
                    elif CFG["midchunks"] == 2:
                        chunks = [(0, 32), (32, OW)]
                    else:
                        chunks = [(0, OW)]
                    for j0, j1 in chunks:
                        nc.sync.dma_start(
                            wv_t[:, j0 * 192 : j1 * 192], wv[i][:, j0 * 192 : j1 * 192]
                        )

                    out_t = opool.tile([128, OUTF], f16, tag="out")

                    if not last:
                        pss = []
                        for h in range(2):
                            js = range(32 * h, min(32 * h + 32, OW))
                            ps = pspool.tile([128, 512], f32, tag="ps")
                            do_bank(ps, js, xsrc, h * 512)
                            pss.append(ps)
                        nc.scalar.copy(out_t[:, :512], pss[0][:])
                        nc.gpsimd.dma_start(
                            out_d[i][:, :512], out_t[:, :512]
                        )
                        nc.vector.tensor_copy(out_t[:, 512:992], pss[1][:, :480])
                        h1q = nc.scalar if CFG.get("mid_h1_act") else nc.gpsimd
                        h1q.dma_start(
                            out_d[i][:, 512:992], out_t[:, 512:992]
                        )
                    else:
                        # Last row: five pipelined j-groups, each with its
                        # OWN psum tile (dependency tracking is tile-
                        # granular, a shared tile would serialize matmuls
                        # behind the previous piece's copy). Copies
                        # alternate ACT/DVE; out DMAs spread over
                        # ACT/Pool/SP queues. Weight chunks arrive in the
                        # same order, so each tiny chain overlaps the
                        # remaining weight stream.
                        # Five pipelined matmul/copy pieces, but only
                        # three out DMAs with >=512B contiguous slices
                        # (sub-512B descriptors pay a 2x latency penalty).
                        # Each DMA is emitted after the last copy covering
                        # its range.
                        eng = {"act": nc.scalar, "dve": nc.vector}
                        qeng = {"act": nc.scalar, "pool": nc.gpsimd,
                                "sp": nc.sync}
                        for j0, j1, ce, dma in CFG["l_pieces"]:
                            f0, f1 = (j0 >> 1) * 32, (j1 >> 1) * 32
                            psn = pspool.tile(
                                [128, 512], f32, tag="ps", name="psn"
                            )
                            do_bank(psn, range(j0, j1), xsrc, f0)
                            if ce == "act":
                                eng[ce].copy(out_t[:, f0:f1], psn[:, : f1 - f0])
                            else:
                                eng[ce].tensor_copy(
                                    out_t[:, f0:f1], psn[:, : f1 - f0]
                                )
                            if dma is not None:
                                g0, g1, q = dma
                                qeng[q].dma_start(
                                    out_d[i][:, g0:g1], out_t[:, g0:g1]
                                )


    nc.compile()
    return nc


def _get_nc():
    if "nc" not in _NC_CACHE:
        _NC_CACHE["nc"] = _build_nc()
    return _NC_CACHE["nc"]


def _prep_in_maps(x, weights):
    """Rearrange full inputs into the per-core SBUF-ready layouts."""
    import ml_dtypes

    f8 = ml_dtypes.float8_e3m4
    x = np.asarray(x, dtype=np.float32)
    weights = np.asarray(weights, dtype=np.float32)

    # x image, padded rows: xtp[h, c, w, b], h in [0, 66)
    xt = x.transpose(2, 1, 3, 0)  # (H, C, W, B)
    xtp = np.zeros((H + 2, C, W, B), np.float16)
    xtp[:H] = xt

    # weights: wt[u, c, i, j, v, o] (u-major partitions), padded i -> 64
    wt = weights.transpose(4, 1, 2, 3, 5, 0)  # (3, C, OH, OW, 3, O)
    wtp = np.zeros((3, C, PADH, OW, 3, O), f8)
    wtp[:, :, :OH] = wt.astype(f8)

    in_maps = []
    for c0 in range(NCORES):
        xi = np.ascontiguousarray(
            xtp[c0 * RPC : c0 * RPC + NXROW].reshape(NXROW, C, XF)
        )
        wvc = np.ascontiguousarray(
            wtp[:, :, c0 * RPC : (c0 + 1) * RPC]
            .transpose(2, 0, 1, 3, 4, 5)
            .reshape(RPC, KP, WF)
        )
        in_maps.append({"xv": xi, "wv": wvc})
    return in_maps


def kernel(x, weights, biases):
    from concourse import bass_utils

    nc = _get_nc()
    in_maps = _prep_in_maps(x, weights)

    res = bass_utils.run_bass_kernel_spmd(
        nc, in_maps, core_ids=list(range(NCORES)), trace=TRACE
    )
    LAST_RESULT["exec_time_ns"] = res.exec_time_ns
    LAST_RESULT["mean_exec_time_ns"] = res.mean_exec_time_ns
    LAST_RESULT["trace"] = res.instructions_and_trace

    full = np.zeros((B, O, PADH, OW), np.float32)
    for c0 in range(NCORES):
        arr = res.results[c0]["out"]  # (RPC, 128, OUTF) f16
        # partition p = (j%2)*64 + o ; free f = (j//2)*32 + b
        a = arr.astype(np.float32).reshape(RPC, 2, O, OW // 2, B)
        a = a.transpose(4, 2, 0, 3, 1).reshape(B, O, RPC, OW)
        full[:, :, c0 * RPC : (c0 + 1) * RPC, :] = a
    out = full[:, :, :OH, :OW]
    out = out + np.asarray(biases, dtype=np.float32)[None]
    return np.ascontiguousarray(out)
